# revision 2
# baseline (speedup 1.0000x reference)
"""Dense MoE layer on 8 NeuronCores, expert-parallel, mixed fp16/fp8.

Math per token t (identical to the reference, all experts dense):
    gates = softmax(x @ Wg + bg);  h_e = gelu(x @ W1[e] + b1[e])
    y_e = h_e @ W2[e] + b2[e];     out = sum_e gates[:,e] * y_e

Sharding: expert-parallel -- core e computes gates[:,e] * y_e for its expert
on all 4096 tokens; the host sums the 8 fp16 partial outputs.

Device dataflow per core, in 8 chunks of 512 tokens (transposed layout so
both matmuls consume natural weight layouts, no on-chip transposes):
    hT = gelu(W1^T @ xT + b1)   fp16 x fp16, lhsT = W1 tiles
    yT = (W2^T @ hT + b2) * G   12 of 16 k-tiles fp16; 4 k-tiles as 2
                                fp8e4m3 DoubleRow matmuls (K=256 each,
                                measured ~2x the bf16/fp16 rate)
Per-expert gate without cross-partition softmax:
    G[t] = 1 / sum_k exp((Wg_k - Wg_e) . x_t + (bg_k - bg_e))
computed as 8 accumulating fp16 matmuls (M=8) + exp + a ones[8,128] matmul
that reduces over E while broadcasting to 128 partitions + DVE reciprocal.

fp8 path: the 4 highest h-tiles' gelu evictions write e4m3 directly (h is
unscaled; |h| <= ~5 sits in e4m3 range, denormal flush below 2^-9 is
negligible); W2 rows for those tiles are host-quantized at x128 to stay
clear of e4m3 denormals.  All 16 DoubleRow matmuls of a chunk run
back-to-back (one bf16->DR LDWEIGHTS transition per chunk instead of 8),
each DR partial accumulates in its own PSUM bank rotation and is evicted
via ACT (x 1/128) to one of 8 SBUF buffers, then merged with the fp16
partial and the gate in two DVE ops.  Measured rel-l2 vs the fp32
reference: 1.8813e-2 (deterministic; gate is 2e-2).  fp16-only variant
measures 5.1e-4 at ~+14us.

DMA: all tensors host-packed so every SBUF tile is a fat contiguous DRAM
slab per partition (128 big descriptors, not 1024x256B strided rows --
strided tile loads measured ~2-3x slower).  Weights alternate whole tiles
across both HWDGE rings (scalar+sync) in consumption order; x chunk 0 is
split across both rings; later x chunks ride sync; small aux via gpsimd
SWDGE; outputs via gpsimd (last chunk via the by-then-idle sync ring).
Chunk 0 is at the shared-HBM roofline (all 8 cores fetch ~10MB in the
first ~30us), so a few us of PE stall there varies run to run.

Measured on trn2 (8 cores): ~469-471us HW exec (baseline 486us), PE busy
~438us of which ~432us is the matmul stream floor.
"""

import numpy as np
import ml_dtypes

D, E, H = 1024, 8, 2048
B, S = 2, 2048
T = B * S
TC = 512
NCH = T // TC
P = 128
ND = D // P          # 8
NH = H // P          # 16
NHB = 12             # layer-2 h-tiles kept in fp16
NPAIR = (NH - NHB) // 2
W2DR_SCALE = 128.0

AUXF_COLS = NH + ND + 1

LAST_RESULTS = None
_NC_CACHE = None


def _build():
    import concourse.bacc as bacc
    import concourse.bass as bass
    import concourse.mybir as mybir
    import concourse.tile as tile

    f32 = mybir.dt.float32
    f16 = mybir.dt.float16
    f8e4 = mybir.dt.float8e4
    AF = mybir.ActivationFunctionType
    OP = mybir.AluOpType
    PSUM = bass.MemorySpace.PSUM
    DR = mybir.MatmulPerfMode.DoubleRow

    nc = bacc.Bacc(None)
    xT16 = nc.dram_tensor("xT16", [P, NCH, ND, TC], f16, kind="ExternalInput")
    w1 = nc.dram_tensor("w1", [P, NH, ND, P], f16, kind="ExternalInput")
    w2 = nc.dram_tensor("w2", [P, ND, NHB, P], f16, kind="ExternalInput")
    w2dr = nc.dram_tensor("w2dr", [P, NPAIR, 2, ND, P], f8e4, kind="ExternalInput")
    auxf = nc.dram_tensor("auxf", [P, AUXF_COLS], f32, kind="ExternalInput")
    wg = nc.dram_tensor("wg", [P, ND, E], f16, kind="ExternalInput")
    yT = nc.dram_tensor("yT", [D, T], f16, kind="ExternalOutput")

    with tile.TileContext(nc) as tc:
        with (
            tc.tile_pool(name="wts", bufs=1) as wts,
            tc.tile_pool(name="xin", bufs=2) as xin,
            tc.tile_pool(name="hb", bufs=1) as hb,
            tc.tile_pool(name="yout", bufs=3) as yout,
            tc.tile_pool(name="gate", bufs=2) as gate,
            tc.tile_pool(name="php", bufs=2, space=PSUM) as php,
            tc.tile_pool(name="pyp", bufs=2, space=PSUM) as pyp,
            tc.tile_pool(name="pdr", bufs=2, space=PSUM) as pdr,
            tc.tile_pool(name="pgp", bufs=1, space=PSUM) as pgp,
            tc.tile_pool(name="pSp", bufs=1, space=PSUM) as pSp,
        ):
            w1s = wts.tile([P, NH, ND, P], f16)        # w1s[p, ht, dt, hc]
            w2s = wts.tile([P, ND, NHB, P], f16)       # w2s[p, dt, ht, dc]
            w2ds = wts.tile([P, NPAIR, 2, ND, P], f8e4)  # [p, q, lane, dt, dc]
            axf = wts.tile([P, AUXF_COLS], f32)
            wgs = wts.tile([P, ND, E], f16)
            ones8 = wts.tile([E, P], f16)

            b1s = axf[:, 0:NH]
            b2s = axf[:, NH : NH + ND]
            bgs = axf[0:E, NH + ND : NH + ND + 1]

            nc.gpsimd.memset(ones8[:], 1.0)
            nc.gpsimd.dma_start(axf[:], auxf[:])
            nc.gpsimd.dma_start(wgs[:], wg[:])

            xcs = [
                xin.tile([P, ND, TC], f16, tag="xc16", name=f"xc{c}")
                for c in range(NCH)
            ]

            nc.scalar.dma_start(xcs[0][:, 0:4], xT16[:, 0, 0:4])
            nc.sync.dma_start(xcs[0][:, 4:8], xT16[:, 0, 4:8])
            # weights alternate whole tiles across both HWDGE rings
            for ht in range(0, NH, 2):
                nc.scalar.dma_start(w1s[:, ht], w1[:, ht])
                nc.sync.dma_start(w1s[:, ht + 1], w1[:, ht + 1])
            for dt in range(0, ND, 2):
                nc.scalar.dma_start(w2s[:, dt], w2[:, dt])
                nc.sync.dma_start(w2s[:, dt + 1], w2[:, dt + 1])
            nc.scalar.dma_start(w2ds[:], w2dr[:])

            def gate_block(xc):
                pg2 = pgp.tile([E, TC], f32, tag="pg")
                for dt in range(ND):
                    nc.tensor.matmul(
                        pg2[:],
                        wgs[:, dt, :],
                        xc[:, dt, :],
                        start=(dt == 0),
                        stop=(dt == ND - 1),
                    )
                ed = gate.tile([E, TC], f16, tag="ed")
                nc.scalar.activation(ed[:], pg2[:], AF.Exp, bias=bgs, scale=1.0)
                pS = pSp.tile([P, TC], f32, tag="pS")
                nc.tensor.matmul(pS[:], ones8[:], ed[:])
                G = gate.tile([P, TC], f32, tag="G")
                nc.vector.reciprocal(G[:], pS[:])
                return G

            for c in range(NCH):
                cs = slice(c * TC, (c + 1) * TC)
                xc = xcs[c]
                if c > 0:
                    nc.sync.dma_start(xc[:], xT16[:, c])

                # --- hT = Gelu(W1^T @ xT + b1) ---
                # chunk 0: spread the gate matmuls between the early h-groups
                # (they only need x) so the streaming w1 tiles get ~200ns of
                # extra slack per group; other chunks: gate after the h-phase
                if c == 0:
                    pg2 = pgp.tile([E, TC], f32, tag="pg")
                hbuf = hb.tile([P, NHB, TC], f16, tag="hbuf")
                h8 = hb.tile([P, NPAIR, 2, TC], f8e4, tag="h8")
                for ht in range(NH):
                    if c == 0 and ht < ND:
                        nc.tensor.matmul(
                            pg2[:],
                            wgs[:, ht, :],
                            xc[:, ht, :],
                            start=(ht == 0),
                            stop=(ht == ND - 1),
                            skip_group_check=True,
                        )
                    ph = php.tile([P, TC], f32, tag="ph")
                    for dt in range(ND):
                        nc.tensor.matmul(
                            ph[:],
                            w1s[:, ht, dt, :],
                            xc[:, dt, :],
                            start=(dt == 0),
                            stop=(dt == ND - 1),
                        )
                    if ht < NHB:
                        hdst = hbuf[:, ht, :]
                    else:
                        hdst = h8[:, (ht - NHB) // 2, (ht - NHB) % 2, :]
                    nc.scalar.activation(
                        hdst, ph[:], AF.Gelu,
                        bias=b1s[:, ht : ht + 1], scale=1.0,
                    )

                if c == 0:
                    ed = gate.tile([E, TC], f16, tag="ed")
                    nc.scalar.activation(ed[:], pg2[:], AF.Exp, bias=bgs, scale=1.0)
                    pS = pSp.tile([P, TC], f32, tag="pS")
                    nc.tensor.matmul(pS[:], ones8[:], ed[:])
                    G = gate.tile([P, TC], f32, tag="G")
                    nc.vector.reciprocal(G[:], pS[:])
                else:
                    G = gate_block(xc)

                # --- yT = ((W2f^T @ hT) + (W2dr^T @ h8)/128 + b2) * G ---
                # all fp8 DR pairs first: 16 back-to-back DR matmuls pay the
                # bf16->DR LDWEIGHTS transition once per chunk, not per dt
                pdsbs = []
                for dt in range(ND):
                    pd = pdr.tile([P, TC], f32, tag="pydr")
                    for q in range(NPAIR):
                        nc.tensor.matmul(
                            pd[:],
                            w2ds[:, q, :, dt, :],
                            h8[:, q, :, :],
                            start=(q == 0),
                            stop=(q == NPAIR - 1),
                            perf_mode=DR,
                        )
                    pdsb = yout.tile(
                        [P, TC], f32, tag="pdsb", bufs=ND, name=f"pdsb{dt}"
                    )
                    nc.scalar.activation(
                        pdsb[:], pd[:], AF.Copy, scale=1.0 / W2DR_SCALE,
                    )
                    pdsbs.append(pdsb)
                for dt in range(ND):
                    py = pyp.tile([P, TC], f32, tag="py")
                    for ht in range(NHB):
                        nc.tensor.matmul(
                            py[:],
                            w2s[:, dt, ht, :],
                            hbuf[:, ht, :],
                            start=(ht == 0),
                            stop=(ht == NHB - 1),
                        )
                    ytmp = yout.tile([P, TC], f32, tag="ytmp")
                    nc.vector.scalar_tensor_tensor(
                        ytmp[:], py[:], b2s[:, dt : dt + 1], pdsbs[dt][:],
                        op0=OP.add, op1=OP.add,
                    )
                    yt = yout.tile([P, TC], f16, tag="yt")
                    nc.vector.scalar_tensor_tensor(
                        yt[:], ytmp[:], 0.0, G[:],
                        op0=OP.add, op1=OP.mult,
                    )
                    out_eng = nc.sync if c == NCH - 1 else nc.gpsimd
                    out_eng.dma_start(yT[dt * P : (dt + 1) * P, cs], yt[:])

    nc.finalize()
    return nc


def kernel(x, Wg, bg, W1, b1, W2, b2):
    global LAST_RESULTS, _NC_CACHE
    from concourse.bass_utils import run_bass_kernel_spmd

    x = np.asarray(x, dtype=np.float32)
    Wg = np.asarray(Wg, dtype=np.float32)
    bg = np.asarray(bg, dtype=np.float32)
    W1 = np.asarray(W1, dtype=np.float32)
    b1 = np.asarray(b1, dtype=np.float32)
    W2 = np.asarray(W2, dtype=np.float32)
    b2 = np.asarray(b2, dtype=np.float32)

    # xT16[p, c, dt, t'] = x[c*TC+t', dt*P+p]
    xT16 = np.ascontiguousarray(
        x.reshape(NCH, TC, ND, P).transpose(3, 0, 2, 1).astype(np.float16)
    )

    in_maps = []
    for e in range(E):
        wgp = Wg - Wg[:, e : e + 1]
        bgp = bg - bg[e]

        auxf = np.zeros((P, AUXF_COLS), dtype=np.float32)
        auxf[:, 0:NH] = b1[e].reshape(NH, P).T
        auxf[:, NH : NH + ND] = b2[e].reshape(ND, P).T
        auxf[0:E, NH + ND] = bgp

        wg16 = np.ascontiguousarray(
            wgp.reshape(ND, P, E).transpose(1, 0, 2).astype(np.float16)
        )

        w1t = np.ascontiguousarray(
            W1[e].astype(np.float16).reshape(ND, P, NH, P).transpose(1, 2, 0, 3)
        )
        w2f = W2[e].reshape(NH, P, ND, P)
        w2t = np.ascontiguousarray(
            w2f[:NHB].astype(np.float16).transpose(1, 2, 0, 3)
        )
        # w2dr[p, q, lane, dt, dc] = W2[(NHB+2q+lane)*P+p, dt*P+dc] * scale
        w2d = np.ascontiguousarray(
            (w2f[NHB:] * W2DR_SCALE)
            .reshape(NPAIR, 2, P, ND, P)
            .transpose(2, 0, 1, 3, 4)
            .astype(ml_dtypes.float8_e4m3)
        )

        in_maps.append(
            {
                "xT16": xT16,
                "w1": w1t,
                "w2": w2t,
                "w2dr": w2d,
                "auxf": auxf,
                "wg": wg16,
            }
        )

    if _NC_CACHE is None:
        _NC_CACHE = _build()
    nc = _NC_CACHE

    res = run_bass_kernel_spmd(nc, in_maps, core_ids=list(range(E)))
    LAST_RESULTS = res

    acc = np.zeros((D, T), dtype=np.float32)
    for e in range(E):
        acc += res.results[e]["yT"].astype(np.float32)
    return np.ascontiguousarray(acc.T).reshape(B, S, D)
